# revision 21
# baseline (speedup 1.0000x reference)
"""Multi-head attention (B=2, S=2048, E=768, H=12, D=64) on 8 TRN2 NeuronCores.

Sharding: batch x heads. Core c handles batch c//4, heads (c%4)*3 .. +3
(Megatron-style: Q/K/V weights column-sharded, Wo row-sharded; the out-projection
partial sums of the 4 cores of each batch are reduced on the host, which also
adds the single shared bias bo).

Per-core pipeline (all matmuls bf16 -> PSUM f32):
  phase 0: load x^T via DMA-transpose from DRAM (host pre-converts inputs and
           weights to bf16), project Q^T,K^T ([64,2048] per head, bias folded in
           on the PSUM->SBUF move) and V ([2048,192], all heads packed).
  phase 1: per (q-block of 512, head, q-tile of 128):
           scores[q,k] = Q^T.T @ K^T tiles -> PSUM [128,2048];
           exp on ScalarE PSUM->SBUF bf16 with accum_out = softmax denominators;
           normalize twice on VectorE (bf16->f32 for the DRAM attn output,
           bf16->bf16 for the AV path); DMA-transpose the normalized bf16 attn
           into attn^T slices; AV matmul contracts k over the 16 attn^T tiles;
           after 3 heads, the out-projection accumulates the per-head
           contributions in PSUM and streams partial [2048,768] to DRAM.
"""
import sys
import os

for _p in ("/root/.axon_site", "/root/.axon_site/_ro/trn_rl_repo", "/opt/trn_rl_repo"):
    if os.path.isdir(_p) and _p not in sys.path:
        sys.path.append(_p)

import numpy as np
import ml_dtypes

import concourse.bacc as bacc
import concourse.mybir as mybir
import concourse.tile as tile
from concourse import bass_utils
from concourse.bass import ts

BF16 = mybir.dt.bfloat16
F32 = mybir.dt.float32
AF = mybir.ActivationFunctionType

B, S, E, H, D = 2, 2048, 768, 12, 64
NCORES = 8
HPC = H * B // NCORES  # heads per core = 3
ET = E // 128          # 6 contraction tiles over E
QT = S // 128          # 16 q/k tiles
QB = 4                 # q-tiles per q-block
NQB = QT // QB         # 4 q-blocks

_cached_nc = None


def build_nc():
    global _cached_nc
    if _cached_nc is not None:
        return _cached_nc
    nc = bacc.Bacc("TRN2", target_bir_lowering=False, debug=False, num_devices=NCORES)

    xq_d = nc.dram_tensor("xq", [S, E], BF16, kind="ExternalInput")
    xk_d = nc.dram_tensor("xk", [S, E], BF16, kind="ExternalInput")
    xv_d = nc.dram_tensor("xv", [S, E], BF16, kind="ExternalInput")
    wq_d = nc.dram_tensor("wq", [E, HPC * D], BF16, kind="ExternalInput")
    wk_d = nc.dram_tensor("wk", [E, HPC * D], BF16, kind="ExternalInput")
    wv_d = nc.dram_tensor("wv", [E, HPC * D], BF16, kind="ExternalInput")
    wo_d = nc.dram_tensor("wo", [D, HPC * E], BF16, kind="ExternalInput")
    bq_d = nc.dram_tensor("bq", [D, HPC], F32, kind="ExternalInput")
    bk_d = nc.dram_tensor("bk", [D, HPC], F32, kind="ExternalInput")
    bv_d = nc.dram_tensor("bv", [128, HPC * D], F32, kind="ExternalInput")

    attn_d = nc.dram_tensor("attn", [HPC, S, S], F32, kind="ExternalOutput")
    part_d = nc.dram_tensor("partial", [S, E], F32, kind="ExternalOutput")

    xq_ap, xk_ap, xv_ap = xq_d.ap(), xk_d.ap(), xv_d.ap(),
    attn_ap, part_ap = attn_d.ap(), part_d.ap()

    with tile.TileContext(nc) as tc:
        with (
            tc.tile_pool(name="consts", bufs=1) as consts,
            tc.tile_pool(name="xt", bufs=1) as xtp,
            tc.tile_pool(name="persist", bufs=1) as persist,
            tc.tile_pool(name="expp", bufs=4) as expp,
            tc.tile_pool(name="attnb", bufs=8) as attnbp,
            tc.tile_pool(name="attnT", bufs=6) as attnTp,
            tc.tile_pool(name="smallsb", bufs=8) as smallsb,
            tc.tile_pool(name="outTp", bufs=2) as outTp,
            tc.tile_pool(name="partsb", bufs=2) as partsb,
            tc.tile_pool(name="psbig", bufs=3, space="PSUM") as psbig,
            tc.tile_pool(name="pssm", bufs=2, space="PSUM") as pssm,
        ):
            # ---- constants ----
            wq_sb = consts.tile([128, ET, HPC * D], BF16)
            nc.sync.dma_start(wq_sb[:], wq_d.ap().rearrange("(a p) j -> p a j", p=128))
            wk_sb = consts.tile([128, ET, HPC * D], BF16)
            nc.sync.dma_start(wk_sb[:], wk_d.ap().rearrange("(a p) j -> p a j", p=128))
            wv_sb = consts.tile([128, ET, HPC * D], BF16)
            nc.sync.dma_start(wv_sb[:], wv_d.ap().rearrange("(a p) j -> p a j", p=128))
            wo_sb = consts.tile([D, HPC * E], BF16)
            nc.sync.dma_start(wo_sb[:], wo_d.ap()[:])
            bq_sb = consts.tile([D, HPC], F32)
            nc.sync.dma_start(bq_sb[:], bq_d.ap()[:])
            bk_sb = consts.tile([D, HPC], F32)
            nc.sync.dma_start(bk_sb[:], bk_d.ap()[:])
            bv_sb = consts.tile([128, HPC * D], F32)
            nc.sync.dma_start(bv_sb[:], bv_d.ap()[:])

            # ---- phase 0: projections ----
            qT = []
            kT = []
            for h in range(HPC):
                qT.append(persist.tile([D, S], BF16, name=f"qT{h}", tag=f"qT{h}"))
                kT.append(persist.tile([D, S], BF16, name=f"kT{h}", tag=f"kT{h}"))
            v_sb = persist.tile([128, QT, HPC * D], BF16, name="v_sb", tag="v_sb")

            def project_qk(x_ap, w_sb, b_sb, dst):
                xT = xtp.tile([128, ET, S], BF16, name="xT", tag="xT")
                # one xbar transpose: xT[p, c, s] = x[s, c*128+p]
                nc.scalar.dma_start_transpose(xT[:], x_ap[:])
                for h in range(HPC):
                    for half in range(2):
                        ps = psbig.tile([D, 1024], F32, name="ps_qk", tag="big")
                        for c in range(2):
                            for e in range(ET):
                                nc.tensor.matmul(
                                    out=ps[:, ts(c, 512)],
                                    lhsT=w_sb[:, e, ts(h, D)],
                                    rhs=xT[:, e, ts(half * 2 + c, 512)],
                                    start=(e == 0),
                                    stop=(e == ET - 1),
                                )
                        nc.vector.tensor_scalar_add(
                            dst[h][:, ts(half, 1024)], ps[:], b_sb[:, h : h + 1]
                        )

            project_qk(xq_ap, wq_sb, bq_sb, qT)
            project_qk(xk_ap, wk_sb, bk_sb, kT)

            xvT = xtp.tile([128, ET, S], BF16, name="xvT", tag="xT")
            nc.scalar.dma_start_transpose(xvT[:], xv_ap[:])
            for t in range(QT):
                psv = pssm.tile([128, HPC * D], F32, name="psv", tag="sm")
                for e in range(ET):
                    nc.tensor.matmul(
                        out=psv[:],
                        lhsT=xvT[:, e, ts(t, 128)],
                        rhs=wv_sb[:, e, :],
                        start=(e == 0),
                        stop=(e == ET - 1),
                    )
                nc.vector.tensor_add(v_sb[:, t, :], psv[:], bv_sb[:])

            # ---- phase 1: attention ----
            def emit_scores_tile(qb, h, i):
                qi = qb * QB + i
                ex = expp.tile([128, S], BF16, name="ex")
                dns = []
                for half in range(2):
                    ps = psbig.tile([128, 1024], F32, name="ps_s", tag="big")
                    for c in range(2):
                        nc.tensor.matmul(
                            out=ps[:, ts(c, 512)],
                            lhsT=qT[h][:, ts(qi, 128)],
                            rhs=kT[h][:, ts(half * 2 + c, 512)],
                            start=True,
                            stop=True,
                        )
                    dnh = smallsb.tile([128, 1], F32, name="dnh", tag=f"dn{half}")
                    nc.scalar.activation(
                        ex[:, ts(half, 1024)], ps[:], AF.Exp, accum_out=dnh[:]
                    )
                    dns.append(dnh)
                dn = smallsb.tile([128, 1], F32, name="dn", tag="dn")
                nc.vector.tensor_add(dn[:], dns[0][:], dns[1][:])
                rc = smallsb.tile([128, 1], F32, name="rc", tag="rc")
                nc.vector.reciprocal(rc[:], dn[:])
                ab = attnbp.tile([128, S], BF16, name="ab")
                nc.vector.tensor_scalar_mul(ab[:], ex[:], rc[:])
                # xbar transpose, contiguous dest: aTi[p, t, j] = ab[j, t*128+p]
                aTi = attnTp.tile([128, QT, 128], BF16, name=f"aT{qb}_{h}_{i}", tag="aT")
                nc.sync.dma_start_transpose(aTi[:], ab[:])
                # f32 attn output: gpsimd casting DMA widens bf16 -> f32 in flight
                nc.gpsimd.dma_start(attn_ap[h, ts(qi, 128), :], ab[:])
                return aTi

            oT = {}

            def emit_av(qb, h, i, aTi):
                po = pssm.tile([D, 128], F32, name="po", tag="sm")
                for t in range(QT):
                    nc.tensor.matmul(
                        out=po[:],
                        lhsT=v_sb[:, t, ts(h, D)],
                        rhs=aTi[:, t, :],
                        start=(t == 0),
                        stop=(t == QT - 1),
                    )
                o = outTp.tile([D, 128], BF16, name=f"oT{qb}_{h}_{i}", tag=f"oT{h}_{i}")
                nc.any.tensor_copy(o[:], po[:])
                oT[(qb, h, i)] = o

            def emit_outproj(qb):
                for i in range(QB):
                    qi = qb * QB + i
                    pl = partsb.tile([128, E], F32, name="pl")
                    for c0, w in ((0, 512), (512, 256)):
                        pp = pssm.tile([128, w], F32, name="pp", tag="sm")
                        for h in range(HPC):
                            nc.tensor.matmul(
                                out=pp[:],
                                lhsT=oT[(qb, h, i)][:, :],
                                rhs=wo_sb[:, h * E + c0 : h * E + c0 + w],
                                start=(h == 0),
                                stop=(h == HPC - 1),
                            )
                        nc.any.tensor_copy(pl[:, c0 : c0 + w], pp[:])
                    nc.gpsimd.dma_start(part_ap[ts(qi, 128), :], pl[:])

            # software-pipeline: AV for tile j is emitted with tile j+LAG's
            # scores so the PE FIFO never blocks on transpose completions
            LAG = 4
            tiles = [
                (qb, h, i) for qb in range(NQB) for h in range(HPC) for i in range(QB)
            ]
            aTis = {}
            for j, (qb, h, i) in enumerate(tiles):
                aTis[j] = (qb, h, i, emit_scores_tile(qb, h, i))
                if j >= LAG:
                    pqb, ph, pi, paTi = aTis.pop(j - LAG)
                    emit_av(pqb, ph, pi, paTi)
                    if ph == HPC - 1 and pi == QB - 1:
                        emit_outproj(pqb)
            for j in sorted(aTis):
                pqb, ph, pi, paTi = aTis.pop(j)
                emit_av(pqb, ph, pi, paTi)
                if ph == HPC - 1 and pi == QB - 1:
                    emit_outproj(pqb)

    nc.compile()
    _cached_nc = nc
    return nc


def _prep_in_maps(query, key, value, Wq, bq, Wk, bk, Wv, bv, Wo, bo):
    bf = ml_dtypes.bfloat16
    scale = np.float32(1.0 / np.sqrt(np.float32(D)))
    x_bf = {}
    for name, arr in (("q", query), ("k", key), ("v", value)):
        for b in range(B):
            x_bf[(name, b)] = np.ascontiguousarray(arr[b]).astype(bf)

    in_maps = []
    for c in range(NCORES):
        b = c // (NCORES // B)
        hs = (c % (NCORES // B)) * HPC
        g0, g1 = hs * D, (hs + HPC) * D
        wq_c = np.ascontiguousarray((Wq[g0:g1, :].T * scale)).astype(bf)
        wk_c = np.ascontiguousarray(Wk[g0:g1, :].T).astype(bf)
        wv_c = np.ascontiguousarray(Wv[g0:g1, :].T).astype(bf)
        wo_c = np.concatenate(
            [Wo[:, (hs + h) * D : (hs + h + 1) * D].T for h in range(HPC)], axis=1
        ).astype(bf)
        bq_c = np.stack(
            [bq[(hs + h) * D : (hs + h + 1) * D] * scale for h in range(HPC)], axis=1
        ).astype(np.float32)
        bk_c = np.stack(
            [bk[(hs + h) * D : (hs + h + 1) * D] for h in range(HPC)], axis=1
        ).astype(np.float32)
        bv_c = np.ascontiguousarray(
            np.broadcast_to(bv[g0:g1][None, :], (128, HPC * D))
        ).astype(np.float32)
        in_maps.append(
            {
                "xq": x_bf[("q", b)],
                "xk": x_bf[("k", b)],
                "xv": x_bf[("v", b)],
                "wq": wq_c,
                "wk": wk_c,
                "wv": wv_c,
                "wo": wo_c,
                "bq": bq_c,
                "bk": bk_c,
                "bv": bv_c,
            }
        )
    return in_maps


def run_cores(in_maps, trace=False, tmpdir=None):
    nc = build_nc()
    return bass_utils.run_bass_kernel_spmd(
        nc, in_maps, core_ids=list(range(NCORES)), trace=trace, tmpdir=tmpdir
    )


def kernel(query, key, value, Wq, bq, Wk, bk, Wv, bv, Wo, bo):
    args = [np.asarray(a, dtype=np.float32) for a in
            (query, key, value, Wq, bq, Wk, bk, Wv, bv, Wo, bo)]
    in_maps = _prep_in_maps(*args)
    res = run_cores(in_maps)

    attn_w = np.empty((B, H, S, S), dtype=np.float32)
    out = np.zeros((B, S, E), dtype=np.float32)
    for c in range(NCORES):
        b = c // (NCORES // B)
        hs = (c % (NCORES // B)) * HPC
        attn_w[b, hs : hs + HPC] = res.results[c]["attn"]
        out[b] += res.results[c]["partial"]
    out += np.asarray(bo, dtype=np.float32)
    return out, attn_w


# revision 22
# speedup vs baseline: 1.1119x; 1.1119x over previous
"""Multi-head attention (B=2, S=2048, E=768, H=12, D=64) on 8 TRN2 NeuronCores.

Sharding: batch x heads. Core c handles batch c//4, heads (c%4)*3 .. +3
(Megatron-style: Q/K/V weights column-sharded, Wo row-sharded; the out-projection
partial sums of the 4 cores of each batch are reduced on the host, which also
adds the single shared bias bo).

Per-core pipeline (all matmuls bf16 -> PSUM f32):
  phase 0: load x^T via DMA-transpose from DRAM (host pre-converts inputs and
           weights to bf16), project Q^T,K^T ([64,2048] per head, bias folded in
           on the PSUM->SBUF move) and V ([2048,192], all heads packed).
  phase 1: per (q-block of 512, head, q-tile of 128):
           scores[q,k] = Q^T.T @ K^T tiles -> PSUM [128,2048];
           exp on ScalarE PSUM->SBUF bf16 with accum_out = softmax denominators;
           normalize twice on VectorE (bf16->f32 for the DRAM attn output,
           bf16->bf16 for the AV path); DMA-transpose the normalized bf16 attn
           into attn^T slices; AV matmul contracts k over the 16 attn^T tiles;
           after 3 heads, the out-projection accumulates the per-head
           contributions in PSUM and streams partial [2048,768] to DRAM.
"""
import sys
import os

for _p in ("/root/.axon_site", "/root/.axon_site/_ro/trn_rl_repo", "/opt/trn_rl_repo"):
    if os.path.isdir(_p) and _p not in sys.path:
        sys.path.append(_p)

import numpy as np
import ml_dtypes

import concourse.bacc as bacc
import concourse.mybir as mybir
import concourse.tile as tile
from concourse import bass_utils
from concourse.bass import ts

BF16 = mybir.dt.bfloat16
F32 = mybir.dt.float32
AF = mybir.ActivationFunctionType

B, S, E, H, D = 2, 2048, 768, 12, 64
NCORES = 8
HPC = H * B // NCORES  # heads per core = 3
ET = E // 128          # 6 contraction tiles over E
QT = S // 128          # 16 q/k tiles
QB = 4                 # q-tiles per q-block
NQB = QT // QB         # 4 q-blocks

_cached_nc = None


def build_nc():
    global _cached_nc
    if _cached_nc is not None:
        return _cached_nc
    nc = bacc.Bacc("TRN2", target_bir_lowering=False, debug=False, num_devices=NCORES)

    xq_d = nc.dram_tensor("xq", [S, E], BF16, kind="ExternalInput")
    xk_d = nc.dram_tensor("xk", [S, E], BF16, kind="ExternalInput")
    xv_d = nc.dram_tensor("xv", [S, E], BF16, kind="ExternalInput")
    wq_d = nc.dram_tensor("wq", [E, HPC * D], BF16, kind="ExternalInput")
    wk_d = nc.dram_tensor("wk", [E, HPC * D], BF16, kind="ExternalInput")
    wv_d = nc.dram_tensor("wv", [E, HPC * D], BF16, kind="ExternalInput")
    wo_d = nc.dram_tensor("wo", [D, HPC * E], BF16, kind="ExternalInput")
    bq_d = nc.dram_tensor("bq", [D, HPC], F32, kind="ExternalInput")
    bk_d = nc.dram_tensor("bk", [D, HPC], F32, kind="ExternalInput")
    bv_d = nc.dram_tensor("bv", [128, HPC * D], F32, kind="ExternalInput")

    attn_d = nc.dram_tensor("attn", [HPC, S, S], F32, kind="ExternalOutput")
    part_d = nc.dram_tensor("partial", [S, E], F32, kind="ExternalOutput")

    xq_ap, xk_ap, xv_ap = xq_d.ap(), xk_d.ap(), xv_d.ap(),
    attn_ap, part_ap = attn_d.ap(), part_d.ap()

    with tile.TileContext(nc) as tc:
        with (
            tc.tile_pool(name="consts", bufs=1) as consts,
            tc.tile_pool(name="xt", bufs=1) as xtp,
            tc.tile_pool(name="persist", bufs=1) as persist,
            tc.tile_pool(name="expp", bufs=4) as expp,
            tc.tile_pool(name="attnb", bufs=8) as attnbp,
            tc.tile_pool(name="attnT", bufs=6) as attnTp,
            tc.tile_pool(name="smallsb", bufs=8) as smallsb,
            tc.tile_pool(name="outTp", bufs=2) as outTp,
            tc.tile_pool(name="partsb", bufs=2) as partsb,
            tc.tile_pool(name="psbig", bufs=3, space="PSUM") as psbig,
            tc.tile_pool(name="pssm", bufs=2, space="PSUM") as pssm,
        ):
            # ---- constants ----
            wq_sb = consts.tile([128, ET, HPC * D], BF16)
            nc.sync.dma_start(wq_sb[:], wq_d.ap().rearrange("(a p) j -> p a j", p=128))
            wk_sb = consts.tile([128, ET, HPC * D], BF16)
            nc.sync.dma_start(wk_sb[:], wk_d.ap().rearrange("(a p) j -> p a j", p=128))
            wv_sb = consts.tile([128, ET, HPC * D], BF16)
            nc.sync.dma_start(wv_sb[:], wv_d.ap().rearrange("(a p) j -> p a j", p=128))
            wo_sb = consts.tile([D, HPC * E], BF16)
            nc.sync.dma_start(wo_sb[:], wo_d.ap()[:])
            bq_sb = consts.tile([D, HPC], F32)
            nc.sync.dma_start(bq_sb[:], bq_d.ap()[:])
            bk_sb = consts.tile([D, HPC], F32)
            nc.sync.dma_start(bk_sb[:], bk_d.ap()[:])
            bv_sb = consts.tile([128, HPC * D], F32)
            nc.sync.dma_start(bv_sb[:], bv_d.ap()[:])

            # ---- phase 0: projections ----
            qT = []
            kT = []
            for h in range(HPC):
                qT.append(persist.tile([D, S], BF16, name=f"qT{h}", tag=f"qT{h}"))
                kT.append(persist.tile([D, S], BF16, name=f"kT{h}", tag=f"kT{h}"))
            v_sb = persist.tile([128, QT, HPC * D], BF16, name="v_sb", tag="v_sb")

            def project_qk(x_ap, w_sb, b_sb, dst):
                xT = xtp.tile([128, ET, S], BF16, name="xT", tag="xT")
                # one xbar transpose: xT[p, c, s] = x[s, c*128+p]
                nc.scalar.dma_start_transpose(xT[:], x_ap[:])
                for h in range(HPC):
                    for half in range(2):
                        ps = psbig.tile([D, 1024], F32, name="ps_qk", tag="big")
                        for c in range(2):
                            for e in range(ET):
                                nc.tensor.matmul(
                                    out=ps[:, ts(c, 512)],
                                    lhsT=w_sb[:, e, ts(h, D)],
                                    rhs=xT[:, e, ts(half * 2 + c, 512)],
                                    start=(e == 0),
                                    stop=(e == ET - 1),
                                )
                        nc.vector.tensor_scalar_add(
                            dst[h][:, ts(half, 1024)], ps[:], b_sb[:, h : h + 1]
                        )

            project_qk(xq_ap, wq_sb, bq_sb, qT)
            project_qk(xk_ap, wk_sb, bk_sb, kT)

            xvT = xtp.tile([128, ET, S], BF16, name="xvT", tag="xT")
            nc.scalar.dma_start_transpose(xvT[:], xv_ap[:])
            for t in range(QT):
                psv = pssm.tile([128, HPC * D], F32, name="psv", tag="sm")
                for e in range(ET):
                    nc.tensor.matmul(
                        out=psv[:],
                        lhsT=xvT[:, e, ts(t, 128)],
                        rhs=wv_sb[:, e, :],
                        start=(e == 0),
                        stop=(e == ET - 1),
                    )
                nc.vector.tensor_add(v_sb[:, t, :], psv[:], bv_sb[:])

            # ---- phase 1: attention ----
            def emit_scores_tile(qb, h, i):
                qi = qb * QB + i
                ex = expp.tile([128, S], BF16, name="ex")
                dns = []
                for half in range(2):
                    ps = psbig.tile([128, 1024], F32, name="ps_s", tag="big")
                    for c in range(2):
                        nc.tensor.matmul(
                            out=ps[:, ts(c, 512)],
                            lhsT=qT[h][:, ts(qi, 128)],
                            rhs=kT[h][:, ts(half * 2 + c, 512)],
                            start=True,
                            stop=True,
                        )
                    dnh = smallsb.tile([128, 1], F32, name="dnh", tag=f"dn{half}")
                    nc.scalar.activation(
                        ex[:, ts(half, 1024)], ps[:], AF.Exp, accum_out=dnh[:]
                    )
                    dns.append(dnh)
                dn = smallsb.tile([128, 1], F32, name="dn", tag="dn")
                nc.vector.tensor_add(dn[:], dns[0][:], dns[1][:])
                rc = smallsb.tile([128, 1], F32, name="rc", tag="rc")
                nc.vector.reciprocal(rc[:], dn[:])
                ab = attnbp.tile([128, S], BF16, name="ab")
                nc.vector.tensor_scalar_mul(ab[:], ex[:], rc[:])
                # xbar transpose, contiguous dest: aTi[p, t, j] = ab[j, t*128+p]
                aTi = attnTp.tile([128, QT, 128], BF16, name=f"aT{qb}_{h}_{i}", tag="aT")
                nc.sync.dma_start_transpose(aTi[:], ab[:])
                # f32 attn output: gpsimd casting DMA widens bf16 -> f32 in flight
                nc.gpsimd.dma_start(attn_ap[h, ts(qi, 128), :], ab[:])
                return aTi

            oT = {}

            def emit_av(qb, h, i, aTi):
                po = pssm.tile([D, 128], F32, name="po", tag="sm")
                for t in range(QT):
                    nc.tensor.matmul(
                        out=po[:],
                        lhsT=v_sb[:, t, ts(h, D)],
                        rhs=aTi[:, t, :],
                        start=(t == 0),
                        stop=(t == QT - 1),
                    )
                o = outTp.tile([D, 128], BF16, name=f"oT{qb}_{h}_{i}", tag=f"oT{h}_{i}")
                nc.vector.tensor_copy(o[:], po[:])
                oT[(qb, h, i)] = o

            def emit_outproj(qb):
                for i in range(QB):
                    qi = qb * QB + i
                    pl = partsb.tile([128, E], F32, name="pl")
                    for c0, w in ((0, 512), (512, 256)):
                        pp = pssm.tile([128, w], F32, name="pp", tag="sm")
                        for h in range(HPC):
                            nc.tensor.matmul(
                                out=pp[:],
                                lhsT=oT[(qb, h, i)][:, :],
                                rhs=wo_sb[:, h * E + c0 : h * E + c0 + w],
                                start=(h == 0),
                                stop=(h == HPC - 1),
                            )
                        nc.vector.tensor_copy(pl[:, c0 : c0 + w], pp[:])
                    nc.gpsimd.dma_start(part_ap[ts(qi, 128), :], pl[:])

            # software-pipeline: AV for tile j is emitted with tile j+LAG's
            # scores so the PE FIFO never blocks on transpose completions
            LAG = 3
            tiles = [
                (qb, h, i) for qb in range(NQB) for h in range(HPC) for i in range(QB)
            ]
            aTis = {}
            for j, (qb, h, i) in enumerate(tiles):
                aTis[j] = (qb, h, i, emit_scores_tile(qb, h, i))
                if j >= LAG:
                    pqb, ph, pi, paTi = aTis.pop(j - LAG)
                    emit_av(pqb, ph, pi, paTi)
                    if ph == HPC - 1 and pi == QB - 1:
                        emit_outproj(pqb)
            for j in sorted(aTis):
                pqb, ph, pi, paTi = aTis.pop(j)
                emit_av(pqb, ph, pi, paTi)
                if ph == HPC - 1 and pi == QB - 1:
                    emit_outproj(pqb)

    nc.compile()
    _cached_nc = nc
    return nc


def _prep_in_maps(query, key, value, Wq, bq, Wk, bk, Wv, bv, Wo, bo):
    bf = ml_dtypes.bfloat16
    scale = np.float32(1.0 / np.sqrt(np.float32(D)))
    x_bf = {}
    for name, arr in (("q", query), ("k", key), ("v", value)):
        for b in range(B):
            x_bf[(name, b)] = np.ascontiguousarray(arr[b]).astype(bf)

    in_maps = []
    for c in range(NCORES):
        b = c // (NCORES // B)
        hs = (c % (NCORES // B)) * HPC
        g0, g1 = hs * D, (hs + HPC) * D
        wq_c = np.ascontiguousarray((Wq[g0:g1, :].T * scale)).astype(bf)
        wk_c = np.ascontiguousarray(Wk[g0:g1, :].T).astype(bf)
        wv_c = np.ascontiguousarray(Wv[g0:g1, :].T).astype(bf)
        wo_c = np.concatenate(
            [Wo[:, (hs + h) * D : (hs + h + 1) * D].T for h in range(HPC)], axis=1
        ).astype(bf)
        bq_c = np.stack(
            [bq[(hs + h) * D : (hs + h + 1) * D] * scale for h in range(HPC)], axis=1
        ).astype(np.float32)
        bk_c = np.stack(
            [bk[(hs + h) * D : (hs + h + 1) * D] for h in range(HPC)], axis=1
        ).astype(np.float32)
        bv_c = np.ascontiguousarray(
            np.broadcast_to(bv[g0:g1][None, :], (128, HPC * D))
        ).astype(np.float32)
        in_maps.append(
            {
                "xq": x_bf[("q", b)],
                "xk": x_bf[("k", b)],
                "xv": x_bf[("v", b)],
                "wq": wq_c,
                "wk": wk_c,
                "wv": wv_c,
                "wo": wo_c,
                "bq": bq_c,
                "bk": bk_c,
                "bv": bv_c,
            }
        )
    return in_maps


def run_cores(in_maps, trace=False, tmpdir=None):
    nc = build_nc()
    return bass_utils.run_bass_kernel_spmd(
        nc, in_maps, core_ids=list(range(NCORES)), trace=trace, tmpdir=tmpdir
    )


def kernel(query, key, value, Wq, bq, Wk, bk, Wv, bv, Wo, bo):
    args = [np.asarray(a, dtype=np.float32) for a in
            (query, key, value, Wq, bq, Wk, bk, Wv, bv, Wo, bo)]
    in_maps = _prep_in_maps(*args)
    res = run_cores(in_maps)

    attn_w = np.empty((B, H, S, S), dtype=np.float32)
    out = np.zeros((B, S, E), dtype=np.float32)
    for c in range(NCORES):
        b = c // (NCORES // B)
        hs = (c % (NCORES // B)) * HPC
        attn_w[b, hs : hs + HPC] = res.results[c]["attn"]
        out[b] += res.results[c]["partial"]
    out += np.asarray(bo, dtype=np.float32)
    return out, attn_w
